# revision 22
# baseline (speedup 1.0000x reference)
"""GAT layer kernel for 8 TRN2 NeuronCores (SPMD, full inputs in / full output out).

Math (per reference):
    h   = inputs @ W                     [B,S,N,F]
    f1  = h @ a1 ; f2 = h @ a2           [B,S,N]
    e   = leaky_relu(f1[...,i,None] + f2[...,None,j], 0.2)
    att = softmax over S of where(Adj>0, e, -9e15)
    out = elu(att @ h), Adj

Sharding: core c handles batch b = c//2 and row-half ih = c%2 (i-range of
512 rows of the attention matrix).  Softmax over S (=8) is elementwise
across the 8 s-slices kept in one wide SBUF tile per j-chunk.

Device-side structure (per core):
  - tiles are [j_chunk=128 partitions, (s,i) free]
  - e[j,i] = f1[i]+f2[j] built by ONE K=4 bf16 matmul per (s, j-chunk):
    lhsT rows (1, 1, f2hi, f2lo), rhs rows (f1hi, f1lo, 1, 1); the hi/lo
    bf16 split keeps ~2^-16 relative precision while the PE runs at
    1 cycle/row (an fp32 matmul would be 4 cycles/row).
    f1/f2 = x @ (W@a) are tiny rank-1 projections precomputed on host in
    f64 and fed as bf16 hi/lo row inputs; all O(N^2) work stays on device.
  - exp/leaky-relu on ACT (Prelu alpha verified on HW), p = exp*adj mask
    alternates DVE / GPSIMD per j-chunk for engine balance.
  - fully-masked (i,j) columns handled exactly via +delta trick:
    att = (p + d) / (sum_p + 8d), d = 2^-64  -> exactly 1/8 when all 8
    s-slices are masked (matches reference softmax over eight -9e15).
  - bf16 attention tensor (DVE 2x mode), f32 e/exp input precision.
"""

import numpy as np
import ml_dtypes

import concourse.bass as bass
import concourse.bacc as bacc
import concourse.mybir as mybir
import concourse.tile as tile
from concourse.bass_utils import run_bass_kernel_spmd

F32 = mybir.dt.float32
BF16 = mybir.dt.bfloat16
AF = mybir.ActivationFunctionType
OP = mybir.AluOpType

B, S, N, FIN, FOUT = 4, 8, 1024, 128, 64
NCORES = 8
NI = 512          # i-rows per core
JC = 8            # j chunks of 128
ALPHA = 0.2
DELTA = float(2.0 ** -64)

_NC_CACHE = {}


def _build_nc(reps=1):
    nc = bacc.Bacc("TRN2", target_bir_lowering=False, debug=False,
                   num_devices=NCORES)
    xT = nc.declare_dram_parameter("xT", [S, FIN, N], BF16, isOutput=False)
    adjT = nc.declare_dram_parameter("adjT", [JC, 128, S * NI], BF16,
                                     isOutput=False)
    w = nc.declare_dram_parameter("w", [FIN, FOUT], BF16, isOutput=False)
    lrow = nc.declare_dram_parameter("lrow", [S, 4, N], BF16, isOutput=False)
    rrow = nc.declare_dram_parameter("rrow", [S, 4, NI], BF16, isOutput=False)
    out = nc.declare_dram_parameter("out", [FOUT, S, NI], F32, isOutput=True)

    import contextlib

    with tile.TileContext(nc) as tc:
        rep_ctx = (tc.For_i(0, reps, 1,
                            hint_engines=(mybir.EngineType.PE,
                                          mybir.EngineType.DVE,
                                          mybir.EngineType.Activation,
                                          mybir.EngineType.SP))
                   if reps > 1 else contextlib.nullcontext())
        with rep_ctx, \
             tc.tile_pool(name="const", bufs=1) as constp, \
             tc.tile_pool(name="xs", bufs=2) as xsp, \
             tc.tile_pool(name="adj", bufs=3) as adjp, \
             tc.tile_pool(name="ee", bufs=3) as eep, \
             tc.tile_pool(name="att", bufs=8) as attp, \
             tc.tile_pool(name="tree", bufs=3) as treep, \
             tc.tile_pool(name="elu", bufs=3) as elup:
            w_sb = constp.tile([FIN, FOUT], BF16, tag="w")
            nc.sync.dma_start(w_sb[:], w[:])
            h_sb = [constp.tile([128, 512], BF16, tag=f"h{s}", name=f"h{s}")
                    for s in range(S)]
            # mixed e-gen operand tiles: rows (1,1,f2hi,f2lo) / (f1hi,f1lo,1,1)
            Ls = [constp.tile([4, N], BF16, tag=f"L{s}", name=f"L{s}")
                  for s in range(S)]
            Rs = [constp.tile([4, NI], BF16, tag=f"R{s}", name=f"R{s}")
                  for s in range(S)]
            for s in range(S):
                nc.sync.dma_start(Ls[s][:], lrow[s])
                nc.sync.dma_start(Rs[s][:], rrow[s])

            # ---------- Phase A: h ----------
            with tc.tile_pool(name="psA", bufs=2, space="PSUM") as psA:
                for s in range(S):
                    xs = xsp.tile([FIN, N], BF16, tag="xs")
                    nc.sync.dma_start(xs[:], xT[s])
                    h_ps = psA.tile([128, 512], F32, tag="hps")
                    for jc in range(JC):
                        nc.tensor.matmul(h_ps[:, jc * 64:(jc + 1) * 64],
                                         xs[:, jc * 128:(jc + 1) * 128],
                                         w_sb[:], start=True, stop=True)
                    nc.scalar.activation(h_sb[s][:], h_ps[:], AF.Copy)

            # ---------- Phase B: e -> prelu -> exp -> mask -> softmax ----------
            # First 4 s-slices of phase C (h' accumulation) are interleaved
            # into B: they accumulate into 4 psum banks as each jc's att tile
            # finalizes, so only s=4..7 remain after B.
            att_tiles = []
            hpw = []
            with (
                tc.tile_pool(name="psE", bufs=1, space="PSUM") as psE,
                tc.tile_pool(name="psCw", bufs=1, space="PSUM") as psCw,
            ):
                def emit_hpw(jc):
                    for s in range(4):
                        nc.tensor.matmul(hpw[s][:],
                                         h_sb[s][:, jc * 64:(jc + 1) * 64],
                                         att_tiles[jc][:, s * NI:(s + 1) * NI],
                                         start=(jc == 0), stop=(jc == JC - 1),
                                         skip_group_check=True)

                for jc in range(JC):
                    adj_t = adjp.tile([128, S * NI], BF16, tag="adj")
                    nc.sync.dma_start(adj_t[:], adjT[jc])
                    att_t = attp.tile([128, S * NI], BF16, tag="att")

                    for sg in range(2):
                        e_ps = psE.tile([128, 4 * NI], F32, tag="eps")
                        for s4 in range(4):
                            s = sg * 4 + s4
                            nc.tensor.matmul(
                                e_ps[:, s4 * NI:(s4 + 1) * NI],
                                Ls[s][:, jc * 128:(jc + 1) * 128],
                                Rs[s][:], start=True, stop=True)
                        e_sb = eep.tile([128, 4 * NI], F32, tag="ee")
                        nc.scalar.activation(e_sb[:], e_ps[:], AF.Prelu,
                                             alpha=ALPHA)
                        nc.scalar.activation(
                            att_t[:, sg * 4 * NI:(sg + 1) * 4 * NI],
                            e_sb[:], AF.Exp)
                        if sg == 1 and jc > 0:
                            emit_hpw(jc - 1)

                    # mask (p = exp * adj); alternate engine for balance
                    meng = nc.vector
                    meng.tensor_tensor(att_t[:], att_t[:], adj_t[:],
                                       op=OP.mult)
                    # denominator tree over s (8 slices)
                    t1 = treep.tile([128, 4 * NI], BF16, tag="t1")
                    teng = nc.vector
                    teng.tensor_tensor(t1[:], att_t[:, 0:4 * NI],
                                       att_t[:, 4 * NI:8 * NI], op=OP.add)
                    t2 = treep.tile([128, 2 * NI], BF16, tag="t2")
                    nc.vector.tensor_tensor(t2[:], t1[:, 0:2 * NI],
                                            t1[:, 2 * NI:4 * NI], op=OP.add)
                    dn2 = treep.tile([128, NI], F32, tag="dn2")
                    nc.vector.scalar_tensor_tensor(dn2[:], t2[:, 0:NI],
                                                   8.0 * DELTA, t2[:, NI:2 * NI],
                                                   op0=OP.add, op1=OP.add)
                    rinv = treep.tile([128, NI], F32, tag="rinv")
                    nc.vector.reciprocal_approx_fast(out=rinv[:], in_=dn2[:])
                    rinv_bf = treep.tile([128, NI], BF16, tag="rinvbf")
                    nc.vector.tensor_copy(rinv_bf[:], rinv[:])
                    # att = (p + delta) * rinv: TS add (4x mode) then
                    # broadcast TT multiply (2x mode)
                    nc.vector.tensor_scalar_add(att_t[:], att_t[:], DELTA)
                    nc.vector.tensor_tensor(
                        att_t[:].rearrange("p (s i) -> p s i", s=S),
                        att_t[:].rearrange("p (s i) -> p s i", s=S),
                        rinv_bf[:, None, :].broadcast_to((128, S, NI)),
                        op=OP.mult)
                    att_tiles.append(att_t)
                    if jc == 0:
                        hpw = [psCw.tile([FOUT, NI], F32, tag=f"hpw{s}",
                                         name=f"hpw{s}") for s in range(4)]
                emit_hpw(JC - 1)

                def elu_out(hp_ap, s):
                    r_sb = elup.tile([FOUT, NI], F32, tag="relu", name="relu")
                    nc.scalar.activation(r_sb[:], hp_ap, AF.Relu)
                    m_sb = elup.tile([FOUT, NI], F32, tag="minv", name="minv")
                    nc.vector.tensor_scalar_min(m_sb[:], hp_ap, 0.0)
                    x_sb = elup.tile([FOUT, NI], F32, tag="expm", name="expm")
                    nc.scalar.activation(x_sb[:], m_sb[:], AF.Exp)
                    o_sb = elup.tile([FOUT, NI], F32, tag="osb", name="osb")
                    nc.vector.scalar_tensor_tensor(o_sb[:], x_sb[:], -1.0,
                                                   r_sb[:], op0=OP.add,
                                                   op1=OP.add)
                    nc.sync.dma_start(out[:, s, :], o_sb[:])

                for s in range(4):
                    elu_out(hpw[s][:], s)

            # ---------- Phase C: h' = att @ h, ELU, out (s = 4..7) ----------
            with tc.tile_pool(name="psC", bufs=4, space="PSUM") as psC:
                for s in range(4, S):
                    hp_ps = psC.tile([FOUT, NI], F32, tag="hp")
                    for jc in range(JC):
                        nc.tensor.matmul(hp_ps[:],
                                         h_sb[s][:, jc * 64:(jc + 1) * 64],
                                         att_tiles[jc][:, s * NI:(s + 1) * NI],
                                         start=(jc == 0), stop=(jc == JC - 1))
                    elu_out(hp_ps[:], s)

    nc.finalize()
    return nc


def _get_nc():
    if "nc" not in _NC_CACHE:
        _NC_CACHE["nc"] = _build_nc()
    return _NC_CACHE["nc"]


def _prep_in_maps(inputs, Adj_np, W_np, a_np):
    bf = ml_dtypes.bfloat16
    xT_all = inputs.transpose(0, 1, 3, 2)                    # [B,S,FIN,N]
    xT_bf = np.ascontiguousarray(xT_all.astype(bf))
    w_bf = np.ascontiguousarray(W_np.astype(bf))

    # f = x @ (W @ a) in float64, split to bf16 hi + lo rows
    a64 = a_np.astype(np.float64)
    wa64 = W_np.astype(np.float64) @ np.stack(
        [a64[:FOUT, 0], a64[FOUT:, 0]], axis=1)               # [FIN, 2]
    f_all = inputs.astype(np.float64) @ wa64                  # [B,S,N,2]
    f1 = f_all[..., 0]                                        # [B,S,N]
    f2 = f_all[..., 1]

    def hilo(v):
        hi = v.astype(bf)
        lo = (v - hi.astype(np.float64)).astype(bf)
        return hi, lo

    in_maps = []
    for c in range(NCORES):
        b, ih = divmod(c, 2)
        irange = slice(ih * NI, (ih + 1) * NI)
        adj_sl = Adj_np[b, :, irange, :]                      # [S, NI, N]
        adjT = np.ascontiguousarray(adj_sl.transpose(2, 0, 1))
        adjT = adjT.reshape(JC, 128, S * NI).astype(bf)
        f2hi, f2lo = hilo(f2[b])                              # [S, N]
        f1hi, f1lo = hilo(f1[b][:, irange])                   # [S, NI]
        ones_n = np.ones((S, N), bf)
        ones_i = np.ones((S, NI), bf)
        lrow_a = np.ascontiguousarray(
            np.stack([ones_n, ones_n, f2hi, f2lo], axis=1))
        rrow_a = np.ascontiguousarray(
            np.stack([f1hi, f1lo, ones_i, ones_i], axis=1))
        in_maps.append({
            "xT": xT_bf[b],
            "adjT": adjT,
            "w": w_bf,
            "lrow": lrow_a,
            "rrow": rrow_a,
        })
    return in_maps


def kernel(inputs, Adj, W, a):
    inputs = np.asarray(inputs, dtype=np.float32)
    Adj_np = np.asarray(Adj)
    W_np = np.asarray(W, dtype=np.float32)
    a_np = np.asarray(a, dtype=np.float32)

    in_maps = _prep_in_maps(inputs, Adj_np, W_np, a_np)
    nc = _get_nc()
    res = run_bass_kernel_spmd(nc, in_maps, list(range(NCORES)))

    full = np.empty((B, S, N, FOUT), np.float32)
    for c in range(NCORES):
        b, ih = divmod(c, 2)
        o = res.results[c]["out"]                             # [FOUT, S, NI]
        full[b, :, ih * NI:(ih + 1) * NI, :] = o.transpose(1, 2, 0)
    return full, Adj


# revision 23
# speedup vs baseline: 1.1788x; 1.1788x over previous
"""GAT layer kernel for 8 TRN2 NeuronCores (SPMD, full inputs in / full output out).

Math (per reference):
    h   = inputs @ W                     [B,S,N,F]
    f1  = h @ a1 ; f2 = h @ a2           [B,S,N]
    e   = leaky_relu(f1[...,i,None] + f2[...,None,j], 0.2)
    att = softmax over S of where(Adj>0, e, -9e15)
    out = elu(att @ h), Adj

Sharding: core c handles batch b = c//2 and row-half ih = c%2 (i-range of
512 rows of the attention matrix).  Softmax over S (=8) is elementwise
across the 8 s-slices kept in one wide SBUF tile per j-chunk.

Device-side structure (per core):
  - tiles are [j_chunk=128 partitions, (s,i) free]
  - e[j,i] = f1[i]+f2[j] built by ONE K=4 bf16 matmul per (s, j-chunk):
    lhsT rows (1, 1, f2hi, f2lo), rhs rows (f1hi, f1lo, 1, 1); the hi/lo
    bf16 split keeps ~2^-16 relative precision while the PE runs at
    1 cycle/row (an fp32 matmul would be 4 cycles/row).
    f1/f2 = x @ (W@a) are tiny rank-1 projections precomputed on host in
    f64 and fed as bf16 hi/lo row inputs; all O(N^2) work stays on device.
  - exp/leaky-relu on ACT (Prelu alpha verified on HW), p = exp*adj mask
    alternates DVE / GPSIMD per j-chunk for engine balance.
  - fully-masked (i,j) columns handled exactly via +delta trick:
    att = (p + d) / (sum_p + 8d), d = 2^-64  -> exactly 1/8 when all 8
    s-slices are masked (matches reference softmax over eight -9e15).
  - bf16 attention tensor (DVE 2x mode), f32 e/exp input precision.
"""

import numpy as np
import ml_dtypes

import concourse.bass as bass
import concourse.bacc as bacc
import concourse.mybir as mybir
import concourse.tile as tile
from concourse.bass_utils import run_bass_kernel_spmd

F32 = mybir.dt.float32
BF16 = mybir.dt.bfloat16
AF = mybir.ActivationFunctionType
OP = mybir.AluOpType

B, S, N, FIN, FOUT = 4, 8, 1024, 128, 64
NCORES = 8
NI = 512          # i-rows per core
JC = 8            # j chunks of 128
ALPHA = 0.2
DELTA = float(2.0 ** -64)

_NC_CACHE = {}


def _build_nc(reps=1):
    nc = bacc.Bacc("TRN2", target_bir_lowering=False, debug=False,
                   num_devices=NCORES)
    xT = nc.declare_dram_parameter("xT", [S, FIN, N], BF16, isOutput=False)
    adjT = nc.declare_dram_parameter("adjT", [JC, 128, S * NI], BF16,
                                     isOutput=False)
    w = nc.declare_dram_parameter("w", [FIN, FOUT], BF16, isOutput=False)
    lrow = nc.declare_dram_parameter("lrow", [S, 4, N], BF16, isOutput=False)
    rrow = nc.declare_dram_parameter("rrow", [S, 4, NI], BF16, isOutput=False)
    out = nc.declare_dram_parameter("out", [FOUT, S, NI], F32, isOutput=True)

    import contextlib

    with tile.TileContext(nc) as tc:
        rep_ctx = (tc.For_i(0, reps, 1,
                            hint_engines=(mybir.EngineType.PE,
                                          mybir.EngineType.DVE,
                                          mybir.EngineType.Activation,
                                          mybir.EngineType.SP))
                   if reps > 1 else contextlib.nullcontext())
        with rep_ctx, \
             tc.tile_pool(name="const", bufs=1) as constp, \
             tc.tile_pool(name="xs", bufs=2) as xsp, \
             tc.tile_pool(name="adj", bufs=3) as adjp, \
             tc.tile_pool(name="ee", bufs=3) as eep, \
             tc.tile_pool(name="att", bufs=8) as attp, \
             tc.tile_pool(name="tree", bufs=3) as treep, \
             tc.tile_pool(name="elu", bufs=3) as elup:
            w_sb = constp.tile([FIN, FOUT], BF16, tag="w")
            nc.sync.dma_start(w_sb[:], w[:])
            h_sb = [constp.tile([128, 512], BF16, tag=f"h{s}", name=f"h{s}")
                    for s in range(S)]
            # mixed e-gen operand tiles: rows (1,1,f2hi,f2lo) / (f1hi,f1lo,1,1)
            Ls = [constp.tile([4, N], BF16, tag=f"L{s}", name=f"L{s}")
                  for s in range(S)]
            Rs = [constp.tile([4, NI], BF16, tag=f"R{s}", name=f"R{s}")
                  for s in range(S)]
            for s in range(S):
                nc.sync.dma_start(Ls[s][:], lrow[s])
                nc.sync.dma_start(Rs[s][:], rrow[s])

            # ---------- Phases A+B overlapped (PSUM pools coexist) ----------
            # B's e-gen depends only on L/R rows, so A (h) and B run
            # concurrently; C's first 3 s-slices accumulate in 3 spare banks.
            att_tiles = []
            hpw = []
            with (
                tc.tile_pool(name="psA", bufs=1, space="PSUM") as psA,
                tc.tile_pool(name="psE", bufs=1, space="PSUM") as psE,
                tc.tile_pool(name="psCw", bufs=1, space="PSUM") as psCw,
            ):
                for s in range(S):
                    xs = xsp.tile([FIN, N], BF16, tag="xs")
                    nc.sync.dma_start(xs[:], xT[s])
                    h_ps = psA.tile([128, 512], F32, tag="hps")
                    for jc in range(JC):
                        nc.tensor.matmul(h_ps[:, jc * 64:(jc + 1) * 64],
                                         xs[:, jc * 128:(jc + 1) * 128],
                                         w_sb[:], start=True, stop=True)
                    nc.scalar.activation(h_sb[s][:], h_ps[:], AF.Copy)

                def emit_hpw(jc):
                    for s in range(3):
                        nc.tensor.matmul(hpw[s][:],
                                         h_sb[s][:, jc * 64:(jc + 1) * 64],
                                         att_tiles[jc][:, s * NI:(s + 1) * NI],
                                         start=(jc == 0), stop=(jc == JC - 1),
                                         skip_group_check=True)

                for jc in range(JC):
                    adj_t = adjp.tile([128, S * NI], BF16, tag="adj")
                    nc.sync.dma_start(adj_t[:], adjT[jc])
                    att_t = attp.tile([128, S * NI], BF16, tag="att")

                    for sg in range(2):
                        e_ps = psE.tile([128, 4 * NI], F32, tag="eps")
                        for s4 in range(4):
                            s = sg * 4 + s4
                            nc.tensor.matmul(
                                e_ps[:, s4 * NI:(s4 + 1) * NI],
                                Ls[s][:, jc * 128:(jc + 1) * 128],
                                Rs[s][:], start=True, stop=True)
                        e_sb = eep.tile([128, 4 * NI], F32, tag="ee")
                        nc.scalar.activation(e_sb[:], e_ps[:], AF.Prelu,
                                             alpha=ALPHA)
                        nc.scalar.activation(
                            att_t[:, sg * 4 * NI:(sg + 1) * 4 * NI],
                            e_sb[:], AF.Exp)
                        if sg == 1 and jc > 0:
                            emit_hpw(jc - 1)

                    # mask (p = exp * adj); alternate engine for balance
                    meng = nc.vector
                    meng.tensor_tensor(att_t[:], att_t[:], adj_t[:],
                                       op=OP.mult)
                    # denominator tree over s (8 slices)
                    t1 = treep.tile([128, 4 * NI], BF16, tag="t1")
                    teng = nc.vector
                    teng.tensor_tensor(t1[:], att_t[:, 0:4 * NI],
                                       att_t[:, 4 * NI:8 * NI], op=OP.add)
                    t2 = treep.tile([128, 2 * NI], BF16, tag="t2")
                    nc.vector.tensor_tensor(t2[:], t1[:, 0:2 * NI],
                                            t1[:, 2 * NI:4 * NI], op=OP.add)
                    dn2 = treep.tile([128, NI], F32, tag="dn2")
                    nc.vector.scalar_tensor_tensor(dn2[:], t2[:, 0:NI],
                                                   8.0 * DELTA, t2[:, NI:2 * NI],
                                                   op0=OP.add, op1=OP.add)
                    rinv = treep.tile([128, NI], F32, tag="rinv")
                    nc.vector.reciprocal_approx_fast(out=rinv[:], in_=dn2[:])
                    rinv_bf = treep.tile([128, NI], BF16, tag="rinvbf")
                    nc.vector.tensor_copy(rinv_bf[:], rinv[:])
                    # att = (p + delta) * rinv: TS add (4x mode) then
                    # broadcast TT multiply (2x mode)
                    nc.vector.tensor_scalar_add(att_t[:], att_t[:], DELTA)
                    nc.vector.tensor_tensor(
                        att_t[:].rearrange("p (s i) -> p s i", s=S),
                        att_t[:].rearrange("p (s i) -> p s i", s=S),
                        rinv_bf[:, None, :].broadcast_to((128, S, NI)),
                        op=OP.mult)
                    att_tiles.append(att_t)
                    if jc == 0:
                        hpw = [psCw.tile([FOUT, NI], F32, tag=f"hpw{s}",
                                         name=f"hpw{s}") for s in range(3)]
                emit_hpw(JC - 1)

                def elu_out(hp_ap, s):
                    r_sb = elup.tile([FOUT, NI], F32, tag="relu", name="relu")
                    nc.scalar.activation(r_sb[:], hp_ap, AF.Relu)
                    m_sb = elup.tile([FOUT, NI], F32, tag="minv", name="minv")
                    nc.vector.tensor_scalar_min(m_sb[:], hp_ap, 0.0)
                    x_sb = elup.tile([FOUT, NI], F32, tag="expm", name="expm")
                    nc.scalar.activation(x_sb[:], m_sb[:], AF.Exp)
                    o_sb = elup.tile([FOUT, NI], F32, tag="osb", name="osb")
                    nc.vector.scalar_tensor_tensor(o_sb[:], x_sb[:], -1.0,
                                                   r_sb[:], op0=OP.add,
                                                   op1=OP.add)
                    nc.sync.dma_start(out[:, s, :], o_sb[:])

                for s in range(3):
                    elu_out(hpw[s][:], s)

            # ---------- Phase C: h' = att @ h, ELU, out (s = 4..7) ----------
            with tc.tile_pool(name="psC", bufs=4, space="PSUM") as psC:
                for s in range(3, S):
                    hp_ps = psC.tile([FOUT, NI], F32, tag="hp")
                    for jc in range(JC):
                        nc.tensor.matmul(hp_ps[:],
                                         h_sb[s][:, jc * 64:(jc + 1) * 64],
                                         att_tiles[jc][:, s * NI:(s + 1) * NI],
                                         start=(jc == 0), stop=(jc == JC - 1))
                    elu_out(hp_ps[:], s)

    nc.finalize()
    return nc


def _get_nc():
    if "nc" not in _NC_CACHE:
        _NC_CACHE["nc"] = _build_nc()
    return _NC_CACHE["nc"]


def _prep_in_maps(inputs, Adj_np, W_np, a_np):
    bf = ml_dtypes.bfloat16
    xT_all = inputs.transpose(0, 1, 3, 2)                    # [B,S,FIN,N]
    xT_bf = np.ascontiguousarray(xT_all.astype(bf))
    w_bf = np.ascontiguousarray(W_np.astype(bf))

    # f = x @ (W @ a) in float64, split to bf16 hi + lo rows
    a64 = a_np.astype(np.float64)
    wa64 = W_np.astype(np.float64) @ np.stack(
        [a64[:FOUT, 0], a64[FOUT:, 0]], axis=1)               # [FIN, 2]
    f_all = inputs.astype(np.float64) @ wa64                  # [B,S,N,2]
    f1 = f_all[..., 0]                                        # [B,S,N]
    f2 = f_all[..., 1]

    def hilo(v):
        hi = v.astype(bf)
        lo = (v - hi.astype(np.float64)).astype(bf)
        return hi, lo

    in_maps = []
    for c in range(NCORES):
        b, ih = divmod(c, 2)
        irange = slice(ih * NI, (ih + 1) * NI)
        adj_sl = Adj_np[b, :, irange, :]                      # [S, NI, N]
        adjT = np.ascontiguousarray(adj_sl.transpose(2, 0, 1))
        adjT = adjT.reshape(JC, 128, S * NI).astype(bf)
        f2hi, f2lo = hilo(f2[b])                              # [S, N]
        f1hi, f1lo = hilo(f1[b][:, irange])                   # [S, NI]
        ones_n = np.ones((S, N), bf)
        ones_i = np.ones((S, NI), bf)
        lrow_a = np.ascontiguousarray(
            np.stack([ones_n, ones_n, f2hi, f2lo], axis=1))
        rrow_a = np.ascontiguousarray(
            np.stack([f1hi, f1lo, ones_i, ones_i], axis=1))
        in_maps.append({
            "xT": xT_bf[b],
            "adjT": adjT,
            "w": w_bf,
            "lrow": lrow_a,
            "rrow": rrow_a,
        })
    return in_maps


def kernel(inputs, Adj, W, a):
    inputs = np.asarray(inputs, dtype=np.float32)
    Adj_np = np.asarray(Adj)
    W_np = np.asarray(W, dtype=np.float32)
    a_np = np.asarray(a, dtype=np.float32)

    in_maps = _prep_in_maps(inputs, Adj_np, W_np, a_np)
    nc = _get_nc()
    res = run_bass_kernel_spmd(nc, in_maps, list(range(NCORES)))

    full = np.empty((B, S, N, FOUT), np.float32)
    for c in range(NCORES):
        b, ih = divmod(c, 2)
        o = res.results[c]["out"]                             # [FOUT, S, NI]
        full[b, :, ih * NI:(ih + 1) * NI, :] = o.transpose(1, 2, 0)
    return full, Adj
